# revision 9
# baseline (speedup 1.0000x reference)
"""Trainium2 Bass kernel for nn_ErrorMinimizationLoss.

Computes: loss = sum(LOSS_SUM[idx] for idx < 64) / max(sum(CNT[idx] for idx < 64), 1)
over codon_indices [16, 8192] int64 (values 0..65; >=64 masked out).
codon_embeddings is unused by the reference arithmetic and is never touched.

Strategy (data-parallel over 8 NeuronCores, 16384 indices per core):
  The scalar output only depends on the 64-bin histogram of the indices, so the
  device computes histograms and the host applies the two 64-entry tables.

  Device layout (per core), "dup-8" scheme: the 16384 indices are split into 16
  groups of 1024; SBUF row p holds group (p % 16), duplicated 8x across rows
  (copy k = p // 16), with k pre-subtracted on the host. Then 8
  tensor_scalar(is_equal, imm 8j) ops on the Vector engine with fused
  accum_out reductions produce red[p, j] = count(idx == 8j + p//16) over group
  p % 16. Invalid indices (64, 65) match no bin -> the validity mask is free.

  Staged start: the input lands via two DMAs (columns [0:H) then [H:W), with
  separate completion semaphores - they can finish out of order); the first
  NHALF ops run as two half-width passes (separate accum slots) so compute
  begins before the second DMA completes. All ordering uses the DMAs' own
  completion semaphores. (A drain+sem_inc completion signal simulates faster
  but returns wrong results on real hardware - the sync-queue drain does not
  cover in-flight HWDGE transfers.) One merged output DMA [128, O+NHALF] f32.

  Host: unscramble red -> 64-bin histogram, dot with tables, final division.
"""
import numpy as np
import ml_dtypes

import concourse.bass as bass
import concourse.mybir as mybir
from concourse.bass_utils import run_bass_kernel_spmd

# ---------------------------------------------------------------- tables ----
AA_PROPS = {'A': (1.8, 88.6, 0.0, 0.0), 'R': (-4.5, 173.4, 52.0, 1.0), 'N': (-3.5, 114.1, 3.38, 0.0), 'D': (-3.5, 111.1, 49.7, -1.0), 'C': (2.5, 108.5, 1.48, 0.0), 'Q': (-3.5, 143.8, 3.53, 0.0), 'E': (-3.5, 138.4, 49.9, -1.0), 'G': (-0.4, 60.1, 0.0, 0.0), 'H': (-3.2, 153.2, 51.6, 0.5), 'I': (4.5, 166.7, 0.13, 0.0), 'L': (3.8, 166.7, 0.13, 0.0), 'K': (-3.9, 168.6, 49.5, 1.0), 'M': (1.9, 162.9, 1.43, 0.0), 'F': (2.8, 189.9, 0.35, 0.0), 'P': (-1.6, 112.7, 1.58, 0.0), 'S': (-0.8, 89.0, 1.67, 0.0), 'T': (-0.7, 116.1, 1.66, 0.0), 'W': (-0.9, 227.8, 2.1, 0.0), 'Y': (-1.3, 193.6, 1.61, 0.0), 'V': (4.2, 140.0, 0.13, 0.0)}
CODON_TABLE = {'UUU': 'F', 'UUC': 'F', 'UUA': 'L', 'UUG': 'L', 'CUU': 'L', 'CUC': 'L', 'CUA': 'L', 'CUG': 'L', 'AUU': 'I', 'AUC': 'I', 'AUA': 'I', 'AUG': 'M', 'GUU': 'V', 'GUC': 'V', 'GUA': 'V', 'GUG': 'V', 'UCU': 'S', 'UCC': 'S', 'UCA': 'S', 'UCG': 'S', 'CCU': 'P', 'CCC': 'P', 'CCA': 'P', 'CCG': 'P', 'ACU': 'T', 'ACC': 'T', 'ACA': 'T', 'ACG': 'T', 'GCU': 'A', 'GCC': 'A', 'GCA': 'A', 'GCG': 'A', 'UAU': 'Y', 'UAC': 'Y', 'UAA': '*', 'UAG': '*', 'CAU': 'H', 'CAC': 'H', 'CAA': 'Q', 'CAG': 'Q', 'AAU': 'N', 'AAC': 'N', 'AAA': 'K', 'AAG': 'K', 'GAU': 'D', 'GAC': 'D', 'GAA': 'E', 'GAG': 'E', 'UGU': 'C', 'UGC': 'C', 'UGA': '*', 'UGG': 'W', 'CGU': 'R', 'CGC': 'R', 'CGA': 'R', 'CGG': 'R', 'AGU': 'S', 'AGC': 'S', 'AGA': 'R', 'AGG': 'R', 'GGU': 'G', 'GGC': 'G', 'GGA': 'G', 'GGG': 'G'}
WEIGHTS = (0.4, 0.3, 0.2, 0.1)
_P3 = 3


def _build_tables():
    codons = list(CODON_TABLE)
    aa_list = list(AA_PROPS)
    aa_idx = {a: i for i, a in enumerate(aa_list)}
    props = np.array([AA_PROPS[a] for a in aa_list], dtype=np.float64)
    maxes = np.abs(props).max(0)
    scale = np.where(maxes > 0, np.array(WEIGHTS) / np.maximum(maxes, 1e-300), 0.0)
    D = (np.abs(props[:, None, :] - props[None, :, :]) * scale).sum(-1)
    cidx = {c: i for i, c in enumerate(codons)}
    loss_sum = np.zeros(64, np.float64)
    cnt = np.zeros(64, np.int64)
    for ci, codon in enumerate(codons):
        aa = CODON_TABLE[codon]
        if aa == '*':
            continue
        for pos in range(3):
            for nt in 'UCAG':
                if nt == codon[pos]:
                    continue
                ni = cidx[codon[:pos] + nt + codon[pos + 1:]]
                naa = CODON_TABLE[codons[ni]]
                if naa == '*':
                    continue
                d = abs(ci - ni)
                v, t = 0, d
                while t > 0 and t % _P3 == 0:
                    v += 1
                    t //= _P3
                padic = float(_P3) ** (-v) if d > 0 else 0.0
                loss_sum[ci] += (D[aa_idx[aa], aa_idx[naa]] - padic) ** 2
                cnt[ci] += 1
    return loss_sum.astype(np.float32), cnt.astype(np.int32)


LOSS_TBL, CNT_TBL = _build_tables()

# --------------------------------------------------------------- device -----
N_CORES = 8
P = 128              # SBUF partitions
DUP = 8              # data duplication factor
NGRP = P // DUP      # 16 token groups per core
TOK = 16384          # tokens per core
W = TOK // NGRP      # 1024 tokens per group (per-row width)
O = 64 // DUP        # 8 compare ops, all on the vector engine
H = 640              # staged-start split column
NHALF = 2            # first NHALF ops run as two half-width passes

_NC_CACHE = None


def _build_nc():
    nc = bass.Bass()
    idx = nc.dram_tensor("idx", [P, W], mybir.dt.bfloat16, kind="ExternalInput")
    red = nc.dram_tensor("red", [P, O + NHALF], mybir.dt.float32, kind="ExternalOutput")

    with (
        nc.sbuf_tensor("sb_idx", [P, W], mybir.dt.bfloat16) as sb_idx,
        nc.sbuf_tensor("sb_scr", [P, O + NHALF, W], mybir.dt.bfloat16) as sb_scr,
        nc.sbuf_tensor("sb_red", [P, O + NHALF], mybir.dt.float32) as sb_red,
        nc.semaphore("dma_sem") as dma_sem,
        nc.semaphore("in2_sem") as in2_sem,
        nc.semaphore("v_sem") as v_sem,
        nc.Block() as block,
    ):
        @block.sync
        def _(sync):
            sync.dma_start(sb_idx[:, :H], idx[:, :H]).then_inc(dma_sem, 16)
            sync.dma_start(sb_idx[:, H:], idx[:, H:]).then_inc(in2_sem, 16)
            sync.wait_ge(v_sem, O + NHALF)
            sync.dma_start(red[:], sb_red[:]).then_inc(dma_sem, 16)

        @block.vector
        def _(v):
            v.wait_ge(dma_sem, 16)
            for j in range(NHALF):
                v.tensor_scalar(
                    out=sb_scr[:, j, :H],
                    in0=sb_idx[:, :H],
                    scalar1=float(DUP * j),
                    scalar2=None,
                    op0=mybir.AluOpType.is_equal,
                    op1=mybir.AluOpType.add,
                    accum_out=sb_red[:, j : j + 1],
                ).then_inc(v_sem, 1)
            v.wait_ge(in2_sem, 16)
            for j in range(NHALF):
                v.tensor_scalar(
                    out=sb_scr[:, j, H:],
                    in0=sb_idx[:, H:],
                    scalar1=float(DUP * j),
                    scalar2=None,
                    op0=mybir.AluOpType.is_equal,
                    op1=mybir.AluOpType.add,
                    accum_out=sb_red[:, O + j : O + j + 1],
                ).then_inc(v_sem, 1)
            for j in range(NHALF, O):
                v.tensor_scalar(
                    out=sb_scr[:, j],
                    in0=sb_idx[:],
                    scalar1=float(DUP * j),
                    scalar2=None,
                    op0=mybir.AluOpType.is_equal,
                    op1=mybir.AluOpType.add,
                    accum_out=sb_red[:, j : j + 1],
                ).then_inc(v_sem, 1)

    return nc


def _get_nc():
    global _NC_CACHE
    if _NC_CACHE is None:
        _NC_CACHE = _build_nc()
    return _NC_CACHE


_K_COL = (np.arange(P) // NGRP).astype(np.float32)[:, None]  # [128, 1]: p // NGRP


def _host_layout(core_tokens):
    """core_tokens: [16384] float32 -> [128, W] bf16 dup layout minus row offset."""
    groups = core_tokens.reshape(NGRP, W)
    tiled = np.tile(groups, (DUP, 1))
    return (tiled - _K_COL).astype(ml_dtypes.bfloat16)


def _unscramble(red):
    """red [P, O+NHALF] f32 -> partial hist[64]: c = DUP*j + p//NGRP.

    Slots O..O+NHALF-1 hold the second half-pass counts of ops 0..NHALF-1.
    """
    full = red[:, :O].astype(np.float64).copy()
    full[:, :NHALF] += red[:, O:].astype(np.float64)
    r = full.reshape(DUP, NGRP, O).sum(axis=1)  # [k, j]
    hist = np.zeros(64, np.float64)
    for k in range(DUP):
        hist[DUP * np.arange(O) + k] += r[k]
    return hist


def kernel(codon_embeddings=None, codon_indices=None, _return_bass_results=False, **_ignored):
    idx = np.asarray(codon_indices)
    flat = idx.reshape(-1).astype(np.float32)
    assert flat.size == N_CORES * TOK, f"unexpected index count {flat.size}"

    in_maps = [
        {"idx": _host_layout(flat[c * TOK : (c + 1) * TOK])} for c in range(N_CORES)
    ]
    res = run_bass_kernel_spmd(
        _get_nc(), in_maps, list(range(N_CORES)), trace=_return_bass_results
    )

    hist = np.zeros(64, np.float64)
    for c in range(N_CORES):
        hist += _unscramble(res.results[c]["red"])

    total = float(hist @ LOSS_TBL.astype(np.float64))
    count = float(hist @ CNT_TBL.astype(np.float64))
    out = np.float32(total / max(count, 1.0))
    if _return_bass_results:
        return out, res
    return out


if __name__ == "__main__":
    rng = np.random.default_rng(0)
    fake_idx = rng.integers(0, 66, size=(16, 8192)).astype(np.int64)
    got = kernel(codon_indices=fake_idx)
    hist = np.bincount(fake_idx.ravel(), minlength=66)[:64]
    want = (hist * LOSS_TBL.astype(np.float64)).sum() / max((hist * CNT_TBL).sum(), 1)
    print("kernel:", got, "expected:", want, "relerr:", abs(got - want) / abs(want))


# revision 10
# speedup vs baseline: 1.0003x; 1.0003x over previous
"""Trainium2 Bass kernel for nn_ErrorMinimizationLoss.

Computes: loss = sum(LOSS_SUM[idx] for idx < 64) / max(sum(CNT[idx] for idx < 64), 1)
over codon_indices [16, 8192] int64 (values 0..65; >=64 masked out).
codon_embeddings is unused by the reference arithmetic and is never touched.

Strategy (data-parallel over 8 NeuronCores, 16384 indices per core):
  The scalar output only depends on the 64-bin histogram of the indices, so the
  device computes histograms and the host applies the two 64-entry tables.

  Device layout (per core), "dup-8" scheme: the 16384 indices are split into 16
  groups of 1024; SBUF row p holds group (p % 16), duplicated 8x across rows
  (copy k = p // 16), with k pre-subtracted on the host. Then 8
  tensor_scalar(is_equal, imm 8j) ops on the Vector engine with fused
  accum_out reductions produce red[p, j] = count(idx == 8j + p//16) over group
  p % 16. Invalid indices (64, 65) match no bin -> the validity mask is free.

  Staged start: the input lands via two DMAs (columns [0:H) then [H:W), with
  separate completion semaphores - they can finish out of order); the first
  NHALF ops run as two half-width passes (separate accum slots) so compute
  begins before the second DMA completes. All ordering uses the DMAs' own
  completion semaphores. (A drain+sem_inc completion signal simulates faster
  but returns wrong results on real hardware - the sync-queue drain does not
  cover in-flight HWDGE transfers.) One merged output DMA [128, O+NHALF] f32.

  Host: unscramble red -> 64-bin histogram, dot with tables, final division.
"""
import numpy as np
import ml_dtypes

import concourse.bass as bass
import concourse.mybir as mybir
from concourse.bass_utils import run_bass_kernel_spmd

# ---------------------------------------------------------------- tables ----
AA_PROPS = {'A': (1.8, 88.6, 0.0, 0.0), 'R': (-4.5, 173.4, 52.0, 1.0), 'N': (-3.5, 114.1, 3.38, 0.0), 'D': (-3.5, 111.1, 49.7, -1.0), 'C': (2.5, 108.5, 1.48, 0.0), 'Q': (-3.5, 143.8, 3.53, 0.0), 'E': (-3.5, 138.4, 49.9, -1.0), 'G': (-0.4, 60.1, 0.0, 0.0), 'H': (-3.2, 153.2, 51.6, 0.5), 'I': (4.5, 166.7, 0.13, 0.0), 'L': (3.8, 166.7, 0.13, 0.0), 'K': (-3.9, 168.6, 49.5, 1.0), 'M': (1.9, 162.9, 1.43, 0.0), 'F': (2.8, 189.9, 0.35, 0.0), 'P': (-1.6, 112.7, 1.58, 0.0), 'S': (-0.8, 89.0, 1.67, 0.0), 'T': (-0.7, 116.1, 1.66, 0.0), 'W': (-0.9, 227.8, 2.1, 0.0), 'Y': (-1.3, 193.6, 1.61, 0.0), 'V': (4.2, 140.0, 0.13, 0.0)}
CODON_TABLE = {'UUU': 'F', 'UUC': 'F', 'UUA': 'L', 'UUG': 'L', 'CUU': 'L', 'CUC': 'L', 'CUA': 'L', 'CUG': 'L', 'AUU': 'I', 'AUC': 'I', 'AUA': 'I', 'AUG': 'M', 'GUU': 'V', 'GUC': 'V', 'GUA': 'V', 'GUG': 'V', 'UCU': 'S', 'UCC': 'S', 'UCA': 'S', 'UCG': 'S', 'CCU': 'P', 'CCC': 'P', 'CCA': 'P', 'CCG': 'P', 'ACU': 'T', 'ACC': 'T', 'ACA': 'T', 'ACG': 'T', 'GCU': 'A', 'GCC': 'A', 'GCA': 'A', 'GCG': 'A', 'UAU': 'Y', 'UAC': 'Y', 'UAA': '*', 'UAG': '*', 'CAU': 'H', 'CAC': 'H', 'CAA': 'Q', 'CAG': 'Q', 'AAU': 'N', 'AAC': 'N', 'AAA': 'K', 'AAG': 'K', 'GAU': 'D', 'GAC': 'D', 'GAA': 'E', 'GAG': 'E', 'UGU': 'C', 'UGC': 'C', 'UGA': '*', 'UGG': 'W', 'CGU': 'R', 'CGC': 'R', 'CGA': 'R', 'CGG': 'R', 'AGU': 'S', 'AGC': 'S', 'AGA': 'R', 'AGG': 'R', 'GGU': 'G', 'GGC': 'G', 'GGA': 'G', 'GGG': 'G'}
WEIGHTS = (0.4, 0.3, 0.2, 0.1)
_P3 = 3


def _build_tables():
    codons = list(CODON_TABLE)
    aa_list = list(AA_PROPS)
    aa_idx = {a: i for i, a in enumerate(aa_list)}
    props = np.array([AA_PROPS[a] for a in aa_list], dtype=np.float64)
    maxes = np.abs(props).max(0)
    scale = np.where(maxes > 0, np.array(WEIGHTS) / np.maximum(maxes, 1e-300), 0.0)
    D = (np.abs(props[:, None, :] - props[None, :, :]) * scale).sum(-1)
    cidx = {c: i for i, c in enumerate(codons)}
    loss_sum = np.zeros(64, np.float64)
    cnt = np.zeros(64, np.int64)
    for ci, codon in enumerate(codons):
        aa = CODON_TABLE[codon]
        if aa == '*':
            continue
        for pos in range(3):
            for nt in 'UCAG':
                if nt == codon[pos]:
                    continue
                ni = cidx[codon[:pos] + nt + codon[pos + 1:]]
                naa = CODON_TABLE[codons[ni]]
                if naa == '*':
                    continue
                d = abs(ci - ni)
                v, t = 0, d
                while t > 0 and t % _P3 == 0:
                    v += 1
                    t //= _P3
                padic = float(_P3) ** (-v) if d > 0 else 0.0
                loss_sum[ci] += (D[aa_idx[aa], aa_idx[naa]] - padic) ** 2
                cnt[ci] += 1
    return loss_sum.astype(np.float32), cnt.astype(np.int32)


LOSS_TBL, CNT_TBL = _build_tables()

# --------------------------------------------------------------- device -----
N_CORES = 8
P = 128              # SBUF partitions
DUP = 8              # data duplication factor
NGRP = P // DUP      # 16 token groups per core
TOK = 16384          # tokens per core
W = TOK // NGRP      # 1024 tokens per group (per-row width)
O = 64 // DUP        # 8 compare ops, all on the vector engine
H = 656              # staged-start split column
NHALF = 2            # first NHALF ops run as two half-width passes

_NC_CACHE = None


def _build_nc():
    nc = bass.Bass()
    idx = nc.dram_tensor("idx", [P, W], mybir.dt.bfloat16, kind="ExternalInput")
    red = nc.dram_tensor("red", [P, O + NHALF], mybir.dt.float32, kind="ExternalOutput")

    with (
        nc.sbuf_tensor("sb_idx", [P, W], mybir.dt.bfloat16) as sb_idx,
        nc.sbuf_tensor("sb_scr", [P, O + NHALF, W], mybir.dt.bfloat16) as sb_scr,
        nc.sbuf_tensor("sb_red", [P, O + NHALF], mybir.dt.float32) as sb_red,
        nc.semaphore("dma_sem") as dma_sem,
        nc.semaphore("in2_sem") as in2_sem,
        nc.semaphore("v_sem") as v_sem,
        nc.Block() as block,
    ):
        @block.sync
        def _(sync):
            sync.dma_start(sb_idx[:, :H], idx[:, :H]).then_inc(dma_sem, 16)
            sync.dma_start(sb_idx[:, H:], idx[:, H:]).then_inc(in2_sem, 16)
            sync.wait_ge(v_sem, O + NHALF)
            sync.dma_start(red[:], sb_red[:]).then_inc(dma_sem, 16)

        @block.vector
        def _(v):
            v.wait_ge(dma_sem, 16)
            for j in range(NHALF):
                v.tensor_scalar(
                    out=sb_scr[:, j, :H],
                    in0=sb_idx[:, :H],
                    scalar1=float(DUP * j),
                    scalar2=None,
                    op0=mybir.AluOpType.is_equal,
                    op1=mybir.AluOpType.add,
                    accum_out=sb_red[:, j : j + 1],
                ).then_inc(v_sem, 1)
            v.wait_ge(in2_sem, 16)
            for j in range(NHALF):
                v.tensor_scalar(
                    out=sb_scr[:, j, H:],
                    in0=sb_idx[:, H:],
                    scalar1=float(DUP * j),
                    scalar2=None,
                    op0=mybir.AluOpType.is_equal,
                    op1=mybir.AluOpType.add,
                    accum_out=sb_red[:, O + j : O + j + 1],
                ).then_inc(v_sem, 1)
            for j in range(NHALF, O):
                v.tensor_scalar(
                    out=sb_scr[:, j],
                    in0=sb_idx[:],
                    scalar1=float(DUP * j),
                    scalar2=None,
                    op0=mybir.AluOpType.is_equal,
                    op1=mybir.AluOpType.add,
                    accum_out=sb_red[:, j : j + 1],
                ).then_inc(v_sem, 1)

    return nc


def _get_nc():
    global _NC_CACHE
    if _NC_CACHE is None:
        _NC_CACHE = _build_nc()
    return _NC_CACHE


_K_COL = (np.arange(P) // NGRP).astype(np.float32)[:, None]  # [128, 1]: p // NGRP


def _host_layout(core_tokens):
    """core_tokens: [16384] float32 -> [128, W] bf16 dup layout minus row offset."""
    groups = core_tokens.reshape(NGRP, W)
    tiled = np.tile(groups, (DUP, 1))
    return (tiled - _K_COL).astype(ml_dtypes.bfloat16)


def _unscramble(red):
    """red [P, O+NHALF] f32 -> partial hist[64]: c = DUP*j + p//NGRP.

    Slots O..O+NHALF-1 hold the second half-pass counts of ops 0..NHALF-1.
    """
    full = red[:, :O].astype(np.float64).copy()
    full[:, :NHALF] += red[:, O:].astype(np.float64)
    r = full.reshape(DUP, NGRP, O).sum(axis=1)  # [k, j]
    hist = np.zeros(64, np.float64)
    for k in range(DUP):
        hist[DUP * np.arange(O) + k] += r[k]
    return hist


def kernel(codon_embeddings=None, codon_indices=None, _return_bass_results=False, **_ignored):
    idx = np.asarray(codon_indices)
    flat = idx.reshape(-1).astype(np.float32)
    assert flat.size == N_CORES * TOK, f"unexpected index count {flat.size}"

    in_maps = [
        {"idx": _host_layout(flat[c * TOK : (c + 1) * TOK])} for c in range(N_CORES)
    ]
    res = run_bass_kernel_spmd(
        _get_nc(), in_maps, list(range(N_CORES)), trace=_return_bass_results
    )

    hist = np.zeros(64, np.float64)
    for c in range(N_CORES):
        hist += _unscramble(res.results[c]["red"])

    total = float(hist @ LOSS_TBL.astype(np.float64))
    count = float(hist @ CNT_TBL.astype(np.float64))
    out = np.float32(total / max(count, 1.0))
    if _return_bass_results:
        return out, res
    return out


if __name__ == "__main__":
    rng = np.random.default_rng(0)
    fake_idx = rng.integers(0, 66, size=(16, 8192)).astype(np.int64)
    got = kernel(codon_indices=fake_idx)
    hist = np.bincount(fake_idx.ravel(), minlength=66)[:64]
    want = (hist * LOSS_TBL.astype(np.float64)).sum() / max((hist * CNT_TBL).sum(), 1)
    print("kernel:", got, "expected:", want, "relerr:", abs(got - want) / abs(want))
